# revision 6
# baseline (speedup 1.0000x reference)
"""Trainium2 Bass kernel: per-edge dot product (u_dot_v GNN edge scoring).

score[e] = sum_d h[src[e], d] * h[dst[e], d]

Strategy
--------
Shard the 1.6M edges across 8 NeuronCores (200k each) and replicate the
node table h into every core's HBM. The per-edge row fetch uses the Q7
`dma_gather` extended-ISA instruction (bulk HBM row gather: one descriptor
per row, ~0.34ns/desc generation), which takes int16 indices — so h is
viewed as 4 banks of 32768 rows, and each core's edges are bucketed on the
host by their (src_bank, dst_bank) pair (16 buckets, statically-sized
capacity with an 8-sigma margin; overflow edges — probability ~1e-13 —
fall back to a host-side dot product). Within a bucket every edge's src
row comes from one fixed bank and its dst row from another, so a chunk of
4096 edges needs exactly two dma_gathers. DVE multiplies the gathered src
and dst tiles elementwise and reduces each 128-wide group to the score.
The host then unpermutes the bucket-sorted scores back to edge order.
"""

import math

import numpy as np

N_NODES = 100000
D_FEAT = 128
N_EDGES = 1600000
N_CORES = 8
P = 128
E_CORE = N_EDGES // N_CORES     # 200000

BANK_SHIFT = 15
BANK_SIZE = 1 << BANK_SHIFT     # 32768
N_BANKS = -(-N_NODES // BANK_SIZE)  # 4
BANK_ROWS = [min(BANK_SIZE, N_NODES - b * BANK_SIZE) for b in range(N_BANKS)]

CHUNK = 4096                    # edges per compute chunk
GMAX = 1024                     # max indices per dma_gather call (HW packet limit)


def _bucket_cap(p):
    m = E_CORE * p
    s = math.sqrt(E_CORE * p * (1.0 - p))
    return max(int(math.ceil((m + 8.0 * s) / 128.0)) * 128, 256)


_pb = [r / N_NODES for r in BANK_ROWS]
CAPS = [_bucket_cap(_pb[i] * _pb[j]) for i in range(N_BANKS) for j in range(N_BANKS)]
CAP_BASE = np.concatenate([[0], np.cumsum(CAPS)])[:-1].astype(np.int64)
TOTCAP = int(sum(CAPS))

# Static chunk schedule: (sorted-offset, chunk_size, src_bank, dst_bank)
CHUNKS = []
for _b in range(N_BANKS * N_BANKS):
    _off = int(CAP_BASE[_b])
    _left = CAPS[_b]
    _sb, _db = _b // N_BANKS, _b % N_BANKS
    while _left > 0:
        _c = min(_left, CHUNK)
        CHUNKS.append((_off, _c, _sb, _db))
        _off += _c
        _left -= _c
N_CHUNKS = len(CHUNKS)

_build_cache = {}


def _build(repeats=1):
    """Build + compile the per-core Bass program.

    DRAM tensors (per core):
      h     [100000, 128] f32   ExternalInput (replicated node features)
      idx   [N_CHUNKS, 2, 128, CHUNK/16] i16 ExternalInput
            (bank-local indices, 16-partition-wrapped, replicated x8)
      score [TOTCAP] f32        ExternalOutput (bucket-sorted order)
    """
    if repeats in _build_cache:
        return _build_cache[repeats]

    from contextlib import ExitStack

    import concourse.tile as tile
    from concourse import bacc, mybir

    nc = bacc.Bacc(
        "TRN2", target_bir_lowering=False, debug=False, num_devices=N_CORES
    )
    h_t = nc.dram_tensor(
        "h", [N_NODES, D_FEAT], mybir.dt.float32, kind="ExternalInput"
    )
    idx_t = nc.dram_tensor(
        "idx", [N_CHUNKS, 2, P, CHUNK // 16], mybir.dt.int16, kind="ExternalInput"
    )
    out_t = nc.dram_tensor(
        "score", [TOTCAP], mybir.dt.float32, kind="ExternalOutput"
    )

    with tile.TileContext(nc) as tc:
        with ExitStack() as ctx:
            idx_pool = ctx.enter_context(tc.tile_pool(name="idxp", bufs=4))
            gat_pool = ctx.enter_context(tc.tile_pool(name="gatp", bufs=3))
            sc_pool = ctx.enter_context(tc.tile_pool(name="scp", bufs=4))
            for _ in range(repeats):
                for ci, (off, c, sb, db) in enumerate(CHUNKS):
                    cw = c // 16     # wrapped idx cols
                    cg = c // 128    # gathered rows per partition
                    idx = idx_pool.tile([P, 2 * (CHUNK // 16)], mybir.dt.int16,
                                        tag="idx")
                    nc.sync.dma_start(
                        out=idx[:, : 2 * cw].rearrange("p (two w) -> p two w", two=2),
                        in_=idx_t.ap()[ci, :, :, :cw].rearrange("two p w -> p two w"),
                    )
                    gs = gat_pool.tile([P, CHUNK], mybir.dt.float32, tag="gs")
                    gd = gat_pool.tile([P, CHUNK], mybir.dt.float32, tag="gd")
                    # dma_gather is limited to 1024 indices per call
                    # (64 descriptors per SDMA engine = one packet)
                    for half, (gt, bank) in enumerate(((gs, sb), (gd, db))):
                        lo = bank * BANK_SIZE
                        done = 0
                        while done < c:
                            g = min(GMAX, c - done)
                            nc.gpsimd.dma_gather(
                                out_ap=gt[:, done : done + g].rearrange(
                                    "p (g d) -> p g d", d=D_FEAT
                                ),
                                in_ap=h_t.ap()[lo : lo + BANK_ROWS[bank]],
                                idxs_ap=idx[
                                    :,
                                    half * cw + done // 16 : half * cw
                                    + (done + g) // 16,
                                ],
                                num_idxs=g,
                                num_idxs_reg=g,
                                elem_size=D_FEAT,
                            )
                            done += g
                    nc.vector.tensor_mul(
                        out=gs[:, :c], in0=gs[:, :c], in1=gd[:, :c]
                    )
                    score = sc_pool.tile([P, CHUNK // 128], mybir.dt.float32,
                                         tag="score")
                    nc.vector.tensor_reduce(
                        out=score[:, :cg],
                        in_=gs[:, :c].rearrange("p (g d) -> p g d", d=D_FEAT),
                        axis=mybir.AxisListType.X,
                        op=mybir.AluOpType.add,
                    )
                    nc.sync.dma_start(
                        out=out_t.ap()[off : off + c].rearrange("(g p) -> p g", p=P),
                        in_=score[:, :cg],
                    )

    nc.compile()
    _build_cache[repeats] = nc
    return nc


def _wrap_idx(a):
    """[c] int16 -> [128, c/16]: idx i at [i%16, i//16], replicated x8."""
    w = a.reshape(-1, 16).T  # [16, c/16]
    return np.tile(w, (8, 1))


def _pack_core_inputs(h32, src, dst, core):
    """Bucket-sort this core's edges by (src_bank, dst_bank); build the
    device idx tensor and the inverse mapping for unpermuting scores.

    Returns (in_map, sorted_pos[E_CORE] int64 (-1 => overflow), overflow
    edge list (orig core-local positions)).
    """
    lo = core * E_CORE
    s = src[lo : lo + E_CORE]
    d = dst[lo : lo + E_CORE]
    sb = s >> BANK_SHIFT
    db = d >> BANK_SHIFT
    bucket = (sb * N_BANKS + db).astype(np.int64)
    order = np.argsort(bucket, kind="stable")
    sizes = np.bincount(bucket, minlength=N_BANKS * N_BANKS)

    sidx_sorted = np.zeros(TOTCAP, np.int16)
    didx_sorted = np.zeros(TOTCAP, np.int16)
    sorted_pos = np.full(E_CORE, -1, np.int64)
    overflow = []
    pos = 0
    for b in range(N_BANKS * N_BANKS):
        n = int(sizes[b])
        take = min(n, CAPS[b])
        sel = order[pos : pos + take]
        base = int(CAP_BASE[b])
        sidx_sorted[base : base + take] = (s[sel] & (BANK_SIZE - 1)).astype(
            np.int16
        )
        didx_sorted[base : base + take] = (d[sel] & (BANK_SIZE - 1)).astype(
            np.int16
        )
        sorted_pos[sel] = base + np.arange(take)
        if n > take:
            overflow.extend(order[pos + take : pos + n].tolist())
        pos += n

    idx_arr = np.zeros((N_CHUNKS, 2, P, CHUNK // 16), np.int16)
    for ci, (off, c, _sb, _db) in enumerate(CHUNKS):
        idx_arr[ci, 0, :, : c // 16] = _wrap_idx(sidx_sorted[off : off + c])
        idx_arr[ci, 1, :, : c // 16] = _wrap_idx(didx_sorted[off : off + c])

    return {"h": h32, "idx": idx_arr}, sorted_pos, overflow


def kernel(h, src, dst):
    from concourse.bass_utils import run_bass_kernel_spmd

    nc = _build()
    h32 = np.ascontiguousarray(np.asarray(h, dtype=np.float32))
    src64 = np.asarray(src).astype(np.int64)
    dst64 = np.asarray(dst).astype(np.int64)

    packed = [_pack_core_inputs(h32, src64, dst64, c) for c in range(N_CORES)]
    in_maps = [p[0] for p in packed]
    res = run_bass_kernel_spmd(nc, in_maps, core_ids=list(range(N_CORES)))

    out = np.empty(N_EDGES, np.float32)
    for c in range(N_CORES):
        _, sorted_pos, overflow = packed[c]
        scores_sorted = res.results[c]["score"]
        oc = out[c * E_CORE : (c + 1) * E_CORE]
        valid = sorted_pos >= 0
        oc[valid] = scores_sorted[sorted_pos[valid]]
        if overflow:
            ov = np.asarray(overflow, np.int64)
            gs = src64[c * E_CORE + ov]
            gd = dst64[c * E_CORE + ov]
            oc[ov] = np.einsum("ed,ed->e", h32[gs], h32[gd])
    return out


# revision 9
# speedup vs baseline: 1.6969x; 1.6969x over previous
"""Trainium2 Bass kernel: per-edge dot product (u_dot_v GNN edge scoring).

score[e] = sum_d h[src[e], d] * h[dst[e], d]

Strategy
--------
Shard the 1.6M edges across 8 NeuronCores (200k each) and replicate the
node table h into every core's HBM. The per-edge row fetch uses the Q7
`dma_gather` extended-ISA instruction (bulk HBM row gather: one descriptor
per row, ~0.34ns/desc generation), which takes int16 indices — so h is
viewed as 4 banks of 32768 rows, and each core's edges are bucketed on the
host by their (src_bank, dst_bank) pair (16 buckets, statically-sized
capacity with an 8-sigma margin; overflow edges — probability ~1e-13 —
fall back to a host-side dot product). Within a bucket every edge's src
row comes from one fixed bank and its dst row from another, so a chunk of
4096 edges needs exactly two dma_gathers. DVE multiplies the gathered src
and dst tiles elementwise and reduces each 128-wide group to the score.
The host then unpermutes the bucket-sorted scores back to edge order.
"""

import math

import numpy as np

N_NODES = 100000
D_FEAT = 128
N_EDGES = 1600000
N_CORES = 8
P = 128
E_CORE = N_EDGES // N_CORES     # 200000

BANK_SHIFT = 15
BANK_SIZE = 1 << BANK_SHIFT     # 32768
N_BANKS = -(-N_NODES // BANK_SIZE)  # 4
BANK_ROWS = [min(BANK_SIZE, N_NODES - b * BANK_SIZE) for b in range(N_BANKS)]

CHUNK = 4096                    # edges per compute chunk
GMAX = 1024                     # max indices per dma_gather call (HW packet limit)


def _bucket_cap(p):
    m = E_CORE * p
    s = math.sqrt(E_CORE * p * (1.0 - p))
    return max(int(math.ceil((m + 8.0 * s) / 128.0)) * 128, 256)


_pb = [r / N_NODES for r in BANK_ROWS]
CAPS = [_bucket_cap(_pb[i] * _pb[j]) for i in range(N_BANKS) for j in range(N_BANKS)]
CAP_BASE = np.concatenate([[0], np.cumsum(CAPS)])[:-1].astype(np.int64)
TOTCAP = int(sum(CAPS))

# Static chunk schedule: (sorted-offset, chunk_size, src_bank, dst_bank)
CHUNKS = []
for _b in range(N_BANKS * N_BANKS):
    _off = int(CAP_BASE[_b])
    _left = CAPS[_b]
    _sb, _db = _b // N_BANKS, _b % N_BANKS
    while _left > 0:
        _c = min(_left, CHUNK)
        CHUNKS.append((_off, _c, _sb, _db))
        _off += _c
        _left -= _c
N_CHUNKS = len(CHUNKS)

_build_cache = {}


def _build(repeats=1):
    """Build + compile the per-core Bass program.

    DRAM tensors (per core):
      h     [100000, 128] f32   ExternalInput (replicated node features)
      idx   [N_CHUNKS, 2, 128, CHUNK/16] i16 ExternalInput
            (bank-local indices, 16-partition-wrapped, replicated x8)
      score [TOTCAP] f32        ExternalOutput (bucket-sorted order)
    """
    if repeats in _build_cache:
        return _build_cache[repeats]

    from contextlib import ExitStack

    import concourse.tile as tile
    from concourse import bacc, mybir

    nc = bacc.Bacc(
        "TRN2",
        target_bir_lowering=False,
        debug=False,
        num_devices=N_CORES,
        num_swdge_queues=4,
    )
    h_t = nc.dram_tensor(
        "h", [N_NODES, D_FEAT], mybir.dt.float32, kind="ExternalInput"
    )
    idx_t = nc.dram_tensor(
        "idx", [N_CHUNKS, 2, P, CHUNK // 16], mybir.dt.int16, kind="ExternalInput"
    )
    out_t = nc.dram_tensor(
        "score", [TOTCAP], mybir.dt.float32, kind="ExternalOutput"
    )

    with tile.TileContext(nc) as tc:
        with ExitStack() as ctx:
            idx_pool = ctx.enter_context(tc.tile_pool(name="idxp", bufs=4))
            gat_pool = ctx.enter_context(tc.tile_pool(name="gatp", bufs=4))
            sc_pool = ctx.enter_context(tc.tile_pool(name="scp", bufs=4))
            gather_ctr = 0
            for _ in range(repeats):
                for ci, (off, c, sb, db) in enumerate(CHUNKS):
                    cw = c // 16     # wrapped idx cols
                    cg = c // 128    # gathered rows per partition
                    idx = idx_pool.tile([P, 2 * (CHUNK // 16)], mybir.dt.int16,
                                        tag="idx")
                    nc.sync.dma_start(
                        out=idx[:, : 2 * cw].rearrange("p (two w) -> p two w", two=2),
                        in_=idx_t.ap()[ci, :, :, :cw].rearrange("two p w -> p two w"),
                    )
                    gs = gat_pool.tile([P, CHUNK], mybir.dt.float32, tag="gs")
                    gd = gat_pool.tile([P, CHUNK], mybir.dt.float32, tag="gd")
                    # dma_gather is limited to 1024 indices per call
                    # (64 descriptors per SDMA engine = one packet)
                    for half, (gt, bank) in enumerate(((gs, sb), (gd, db))):
                        lo = bank * BANK_SIZE
                        done = 0
                        while done < c:
                            g = min(GMAX, c - done)
                            nc.gpsimd.dma_gather(
                                out_ap=gt[:, done : done + g].rearrange(
                                    "p (g d) -> p g d", d=D_FEAT
                                ),
                                in_ap=h_t.ap()[lo : lo + BANK_ROWS[bank]],
                                idxs_ap=idx[
                                    :,
                                    half * cw + done // 16 : half * cw
                                    + (done + g) // 16,
                                ],
                                num_idxs=g,
                                num_idxs_reg=g,
                                elem_size=D_FEAT,
                                queue_num=gather_ctr % 4,
                            )
                            gather_ctr += 1
                            done += g
                    nc.vector.tensor_mul(
                        out=gs[:, :c], in0=gs[:, :c], in1=gd[:, :c]
                    )
                    score = sc_pool.tile([P, CHUNK // 128], mybir.dt.float32,
                                         tag="score")
                    nc.vector.tensor_reduce(
                        out=score[:, :cg],
                        in_=gs[:, :c].rearrange("p (g d) -> p g d", d=D_FEAT),
                        axis=mybir.AxisListType.X,
                        op=mybir.AluOpType.add,
                    )
                    nc.sync.dma_start(
                        out=out_t.ap()[off : off + c].rearrange("(g p) -> p g", p=P),
                        in_=score[:, :cg],
                    )

    nc.compile()
    _build_cache[repeats] = nc
    return nc


def _wrap_idx(a):
    """[c] int16 -> [128, c/16]: idx i at [i%16, i//16], replicated x8."""
    w = a.reshape(-1, 16).T  # [16, c/16]
    return np.tile(w, (8, 1))


def _pack_core_inputs(h32, src, dst, core):
    """Bucket-sort this core's edges by (src_bank, dst_bank); build the
    device idx tensor and the inverse mapping for unpermuting scores.

    Returns (in_map, sorted_pos[E_CORE] int64 (-1 => overflow), overflow
    edge list (orig core-local positions)).
    """
    lo = core * E_CORE
    s = src[lo : lo + E_CORE]
    d = dst[lo : lo + E_CORE]
    sb = s >> BANK_SHIFT
    db = d >> BANK_SHIFT
    bucket = (sb * N_BANKS + db).astype(np.int64)
    order = np.argsort(bucket, kind="stable")
    sizes = np.bincount(bucket, minlength=N_BANKS * N_BANKS)

    sidx_sorted = np.zeros(TOTCAP, np.int16)
    didx_sorted = np.zeros(TOTCAP, np.int16)
    sorted_pos = np.full(E_CORE, -1, np.int64)
    overflow = []
    pos = 0
    for b in range(N_BANKS * N_BANKS):
        n = int(sizes[b])
        take = min(n, CAPS[b])
        sel = order[pos : pos + take]
        base = int(CAP_BASE[b])
        sidx_sorted[base : base + take] = (s[sel] & (BANK_SIZE - 1)).astype(
            np.int16
        )
        didx_sorted[base : base + take] = (d[sel] & (BANK_SIZE - 1)).astype(
            np.int16
        )
        sorted_pos[sel] = base + np.arange(take)
        if n > take:
            overflow.extend(order[pos + take : pos + n].tolist())
        pos += n

    idx_arr = np.zeros((N_CHUNKS, 2, P, CHUNK // 16), np.int16)
    for ci, (off, c, _sb, _db) in enumerate(CHUNKS):
        idx_arr[ci, 0, :, : c // 16] = _wrap_idx(sidx_sorted[off : off + c])
        idx_arr[ci, 1, :, : c // 16] = _wrap_idx(didx_sorted[off : off + c])

    return {"h": h32, "idx": idx_arr}, sorted_pos, overflow


def kernel(h, src, dst):
    from concourse.bass_utils import run_bass_kernel_spmd

    nc = _build()
    h32 = np.ascontiguousarray(np.asarray(h, dtype=np.float32))
    src64 = np.asarray(src).astype(np.int64)
    dst64 = np.asarray(dst).astype(np.int64)

    packed = [_pack_core_inputs(h32, src64, dst64, c) for c in range(N_CORES)]
    in_maps = [p[0] for p in packed]
    res = run_bass_kernel_spmd(nc, in_maps, core_ids=list(range(N_CORES)))

    out = np.empty(N_EDGES, np.float32)
    for c in range(N_CORES):
        _, sorted_pos, overflow = packed[c]
        scores_sorted = res.results[c]["score"]
        oc = out[c * E_CORE : (c + 1) * E_CORE]
        valid = sorted_pos >= 0
        oc[valid] = scores_sorted[sorted_pos[valid]]
        if overflow:
            ov = np.asarray(overflow, np.int64)
            gs = src64[c * E_CORE + ov]
            gd = dst64[c * E_CORE + ov]
            oc[ov] = np.einsum("ed,ed->e", h32[gs], h32[gd])
    return out


# revision 14
# speedup vs baseline: 1.8530x; 1.0919x over previous
"""Trainium2 Bass kernel: per-edge dot product (u_dot_v GNN edge scoring).

score[e] = sum_d h[src[e], d] * h[dst[e], d]

Strategy
--------
Shard the 1.6M edges across 8 NeuronCores (200k each) and replicate the
node table h into every core's HBM. The per-edge row fetch uses the Q7
`dma_gather` extended-ISA instruction (bulk HBM row gather: one descriptor
per row, ~0.34ns/desc generation), which takes int16 indices — so h is
viewed as 4 banks of 32768 rows, and each core's edges are bucketed on the
host by their (src_bank, dst_bank) pair (16 buckets, statically-sized
capacity with an 8-sigma margin; overflow edges — probability ~1e-13 —
fall back to a host-side dot product). Within a bucket every edge's src
row comes from one fixed bank and its dst row from another, so a chunk of
4096 edges needs exactly two dma_gathers. DVE multiplies the gathered src
and dst tiles elementwise and reduces each 128-wide group to the score.
The host then unpermutes the bucket-sorted scores back to edge order.
"""

import math

import numpy as np

N_NODES = 100000
D_FEAT = 128
N_EDGES = 1600000
N_CORES = 8
P = 128
E_CORE = N_EDGES // N_CORES     # 200000

BANK_SHIFT = 15
BANK_SIZE = 1 << BANK_SHIFT     # 32768
N_BANKS = -(-N_NODES // BANK_SIZE)  # 4
BANK_ROWS = [min(BANK_SIZE, N_NODES - b * BANK_SIZE) for b in range(N_BANKS)]

CHUNK = 4096                    # edges per compute chunk
GMAX = 1024                     # max indices per dma_gather call (HW packet limit)


def _bucket_cap(p):
    m = E_CORE * p
    s = math.sqrt(E_CORE * p * (1.0 - p))
    return max(int(math.ceil((m + 8.0 * s) / 128.0)) * 128, 256)


_pb = [r / N_NODES for r in BANK_ROWS]
CAPS = [_bucket_cap(_pb[i] * _pb[j]) for i in range(N_BANKS) for j in range(N_BANKS)]
CAP_BASE = np.concatenate([[0], np.cumsum(CAPS)])[:-1].astype(np.int64)
TOTCAP = int(sum(CAPS))

# Static chunk schedule: (sorted-offset, chunk_size, src_bank, dst_bank)
CHUNKS = []
for _b in range(N_BANKS * N_BANKS):
    _off = int(CAP_BASE[_b])
    _left = CAPS[_b]
    _sb, _db = _b // N_BANKS, _b % N_BANKS
    while _left > 0:
        _c = min(_left, CHUNK)
        CHUNKS.append((_off, _c, _sb, _db))
        _off += _c
        _left -= _c
N_CHUNKS = len(CHUNKS)

_build_cache = {}


def _build(repeats=1):
    """Build + compile the per-core Bass program.

    DRAM tensors (per core):
      h     [100000, 128] f32   ExternalInput (replicated node features)
      idx   [N_CHUNKS, 2, 128, CHUNK/16] i16 ExternalInput
            (bank-local indices, 16-partition-wrapped, replicated x8)
      score [TOTCAP] f32        ExternalOutput (bucket-sorted order)
    """
    if repeats in _build_cache:
        return _build_cache[repeats]

    from contextlib import ExitStack

    import concourse.tile as tile
    from concourse import bacc, mybir
    from concourse.tile import add_dep_helper

    nc = bacc.Bacc(
        "TRN2",
        target_bir_lowering=False,
        debug=False,
        num_devices=N_CORES,
        num_swdge_queues=4,
    )
    h_t = nc.dram_tensor(
        "h", [N_NODES, D_FEAT], mybir.dt.float32, kind="ExternalInput"
    )
    idx_t = nc.dram_tensor(
        "idx", [N_CHUNKS, 2, P, CHUNK // 16], mybir.dt.int16, kind="ExternalInput"
    )
    out_t = nc.dram_tensor(
        "score", [TOTCAP], mybir.dt.float32, kind="ExternalOutput"
    )

    with tile.TileContext(nc) as tc:
        with ExitStack() as ctx:
            idx_pool = ctx.enter_context(tc.tile_pool(name="idxp", bufs=4))
            gat_pool = ctx.enter_context(tc.tile_pool(name="gatp", bufs=12))
            sc_pool = ctx.enter_context(tc.tile_pool(name="scp", bufs=4))
            gather_ctr = 0
            prev_gather = None
            for _ in range(repeats):
                for ci, (off, c, sb, db) in enumerate(CHUNKS):
                    cw = c // 16     # wrapped idx cols
                    cg = c // 128    # gathered rows per partition
                    idx = idx_pool.tile([P, 2 * (CHUNK // 16)], mybir.dt.int16,
                                        tag="idx")
                    nc.sync.dma_start(
                        out=idx[:, : 2 * cw].rearrange("p (two w) -> p two w", two=2),
                        in_=idx_t.ap()[ci, :, :, :cw].rearrange("two p w -> p two w"),
                    )
                    score = sc_pool.tile([P, CHUNK // 128], mybir.dt.float32,
                                         tag="score")
                    # dma_gather is limited to 1024 indices per call (64
                    # descriptors per SDMA engine = one packet). Work in
                    # 1024-edge groups — two gathers -> mul -> reduce — so
                    # each DVE op depends on just two DMAs and overlap stays
                    # tight.
                    done = 0
                    while done < c:
                        g = min(GMAX, c - done)
                        ts = gat_pool.tile([P, GMAX], mybir.dt.float32,
                                           tag="ts")
                        td = gat_pool.tile([P, GMAX], mybir.dt.float32,
                                           tag="td")
                        for gt, bank, base in ((ts, sb, 0), (td, db, cw)):
                            gi = nc.gpsimd.dma_gather(
                                out_ap=gt[:, :g].rearrange(
                                    "p (g d) -> p g d", d=D_FEAT
                                ),
                                in_ap=h_t.ap()[
                                    bank * BANK_SIZE : bank * BANK_SIZE
                                    + BANK_ROWS[bank]
                                ],
                                idxs_ap=idx[
                                    :, base + done // 16 : base + (done + g) // 16
                                ],
                                num_idxs=g,
                                num_idxs_reg=g,
                                elem_size=D_FEAT,
                                queue_num=gather_ctr % 4,
                            )
                            # Pin gather issue order = program order so the
                            # scheduler's DMASW lane rotation (8 lanes, by
                            # scheduled Pool-DMA order) stays aligned with the
                            # queue rotation (4 queues, program order) — a
                            # lane may only ever be updated from one queue.
                            if prev_gather is not None:
                                add_dep_helper(gi.ins, prev_gather.ins,
                                               sync=False)
                            prev_gather = gi
                            gather_ctr += 1
                        nc.vector.tensor_mul(
                            out=ts[:, :g], in0=ts[:, :g], in1=td[:, :g]
                        )
                        nc.vector.tensor_reduce(
                            out=score[:, done // 128 : (done + g) // 128],
                            in_=ts[:, :g].rearrange("p (g d) -> p g d", d=D_FEAT),
                            axis=mybir.AxisListType.X,
                            op=mybir.AluOpType.add,
                        )
                        done += g
                    # scores leave on the ACT HWDGE ring so they never queue
                    # behind upcoming idx loads on the SP ring (HWDGE is FIFO
                    # per issuing engine)
                    nc.scalar.dma_start(
                        out=out_t.ap()[off : off + c].rearrange("(g p) -> p g", p=P),
                        in_=score[:, :cg],
                    )

    nc.compile()
    _build_cache[repeats] = nc
    return nc


def _wrap_idx(a):
    """[c] int16 -> [128, c/16]: idx i at [i%16, i//16], replicated x8."""
    w = a.reshape(-1, 16).T  # [16, c/16]
    return np.tile(w, (8, 1))


def _pack_core_inputs(h32, src, dst, core):
    """Bucket-sort this core's edges by (src_bank, dst_bank); build the
    device idx tensor and the inverse mapping for unpermuting scores.

    Returns (in_map, sorted_pos[E_CORE] int64 (-1 => overflow), overflow
    edge list (orig core-local positions)).
    """
    lo = core * E_CORE
    s = src[lo : lo + E_CORE]
    d = dst[lo : lo + E_CORE]
    sb = s >> BANK_SHIFT
    db = d >> BANK_SHIFT
    bucket = (sb * N_BANKS + db).astype(np.int64)
    order = np.argsort(bucket, kind="stable")
    sizes = np.bincount(bucket, minlength=N_BANKS * N_BANKS)

    sidx_sorted = np.zeros(TOTCAP, np.int16)
    didx_sorted = np.zeros(TOTCAP, np.int16)
    sorted_pos = np.full(E_CORE, -1, np.int64)
    overflow = []
    pos = 0
    for b in range(N_BANKS * N_BANKS):
        n = int(sizes[b])
        take = min(n, CAPS[b])
        sel = order[pos : pos + take]
        base = int(CAP_BASE[b])
        sidx_sorted[base : base + take] = (s[sel] & (BANK_SIZE - 1)).astype(
            np.int16
        )
        didx_sorted[base : base + take] = (d[sel] & (BANK_SIZE - 1)).astype(
            np.int16
        )
        sorted_pos[sel] = base + np.arange(take)
        if n > take:
            overflow.extend(order[pos + take : pos + n].tolist())
        pos += n

    idx_arr = np.zeros((N_CHUNKS, 2, P, CHUNK // 16), np.int16)
    for ci, (off, c, _sb, _db) in enumerate(CHUNKS):
        idx_arr[ci, 0, :, : c // 16] = _wrap_idx(sidx_sorted[off : off + c])
        idx_arr[ci, 1, :, : c // 16] = _wrap_idx(didx_sorted[off : off + c])

    return {"h": h32, "idx": idx_arr}, sorted_pos, overflow


def kernel(h, src, dst):
    from concourse.bass_utils import run_bass_kernel_spmd

    nc = _build()
    h32 = np.ascontiguousarray(np.asarray(h, dtype=np.float32))
    src64 = np.asarray(src).astype(np.int64)
    dst64 = np.asarray(dst).astype(np.int64)

    packed = [_pack_core_inputs(h32, src64, dst64, c) for c in range(N_CORES)]
    in_maps = [p[0] for p in packed]
    res = run_bass_kernel_spmd(nc, in_maps, core_ids=list(range(N_CORES)))

    out = np.empty(N_EDGES, np.float32)
    for c in range(N_CORES):
        _, sorted_pos, overflow = packed[c]
        scores_sorted = res.results[c]["score"]
        oc = out[c * E_CORE : (c + 1) * E_CORE]
        valid = sorted_pos >= 0
        oc[valid] = scores_sorted[sorted_pos[valid]]
        if overflow:
            ov = np.asarray(overflow, np.int64)
            gs = src64[c * E_CORE + ov]
            gd = dst64[c * E_CORE + ov]
            oc[ov] = np.einsum("ed,ed->e", h32[gs], h32[gd])
    return out


# revision 19
# speedup vs baseline: 2.7360x; 1.4766x over previous
"""Trainium2 Bass kernel: per-edge dot product (u_dot_v GNN edge scoring).

score[e] = sum_d h[src[e], d] * h[dst[e], d]

Strategy
--------
Shard the 1.6M edges across 8 NeuronCores (200k each) and replicate the
node table h into every core's HBM. The per-edge row fetch uses the Q7
`dma_gather` extended-ISA instruction (bulk HBM row gather: one descriptor
per row, ~0.34ns/desc generation), which takes int16 indices — so h is
viewed as 4 banks of 32768 rows, and each core's edges are bucketed on the
host by their (src_bank, dst_bank) pair (16 buckets, statically-sized
capacity with an 8-sigma margin; overflow edges — probability ~1e-13 —
fall back to a host-side dot product). Within a bucket every edge's src
row comes from one fixed bank and its dst row from another, so a chunk of
4096 edges needs exactly two dma_gathers. DVE multiplies the gathered src
and dst tiles elementwise and reduces each 128-wide group to the score.
The host then unpermutes the bucket-sorted scores back to edge order.
"""

import math

import numpy as np

N_NODES = 100000
D_FEAT = 128
N_EDGES = 1600000
N_CORES = 8
P = 128
E_CORE = N_EDGES // N_CORES     # 200000

BANK_SHIFT = 15
BANK_SIZE = 1 << BANK_SHIFT     # 32768
N_BANKS = -(-N_NODES // BANK_SIZE)  # 4
BANK_ROWS = [min(BANK_SIZE, N_NODES - b * BANK_SIZE) for b in range(N_BANKS)]

CHUNK = 4096                    # edges per compute chunk
GMAX = 1024                     # max indices per dma_gather call (HW packet limit)


def _bucket_cap(p):
    m = E_CORE * p
    s = math.sqrt(E_CORE * p * (1.0 - p))
    return max(int(math.ceil((m + 8.0 * s) / 128.0)) * 128, 256)


_pb = [r / N_NODES for r in BANK_ROWS]
CAPS = [_bucket_cap(_pb[i] * _pb[j]) for i in range(N_BANKS) for j in range(N_BANKS)]
CAP_BASE = np.concatenate([[0], np.cumsum(CAPS)])[:-1].astype(np.int64)
TOTCAP = int(sum(CAPS))

# Static chunk schedule: (sorted-offset, chunk_size, src_bank, dst_bank)
CHUNKS = []
for _b in range(N_BANKS * N_BANKS):
    _off = int(CAP_BASE[_b])
    _left = CAPS[_b]
    _sb, _db = _b // N_BANKS, _b % N_BANKS
    while _left > 0:
        _c = min(_left, CHUNK)
        CHUNKS.append((_off, _c, _sb, _db))
        _off += _c
        _left -= _c
N_CHUNKS = len(CHUNKS)

_build_cache = {}


def _build(repeats=1):
    """Build + compile the per-core Bass program.

    DRAM tensors (per core):
      h     [100000, 128] f32   ExternalInput (replicated node features)
      idx   [N_CHUNKS, 2, 128, CHUNK/16] i16 ExternalInput
            (bank-local indices, 16-partition-wrapped, replicated x8)
      score [TOTCAP] f32        ExternalOutput (bucket-sorted order)
    """
    if repeats in _build_cache:
        return _build_cache[repeats]

    from contextlib import ExitStack

    import concourse.tile as tile
    from concourse import bacc, mybir
    from concourse.tile import add_dep_helper

    nc = bacc.Bacc(
        "TRN2",
        target_bir_lowering=False,
        debug=False,
        num_devices=N_CORES,
        num_swdge_queues=4,
    )
    h_t = nc.dram_tensor(
        "h", [N_NODES, D_FEAT], mybir.dt.float32, kind="ExternalInput"
    )
    # partition-major so one DMA loads the whole thing with 128 big
    # contiguous descriptors
    idx_t = nc.dram_tensor(
        "idx", [P, N_CHUNKS, 2, CHUNK // 16], mybir.dt.int16, kind="ExternalInput"
    )
    # scores stay partition-major too: score[p, j] = sorted score j*128+p
    out_t = nc.dram_tensor(
        "score", [P, TOTCAP // P], mybir.dt.float32, kind="ExternalOutput"
    )

    with tile.TileContext(nc) as tc:
        with ExitStack() as ctx:
            idx_pool = ctx.enter_context(tc.tile_pool(name="idxp", bufs=1))
            gat_pool = ctx.enter_context(tc.tile_pool(name="gatp", bufs=10))
            sc_pool = ctx.enter_context(tc.tile_pool(name="scp", bufs=1))
            IDXW = N_CHUNKS * 2 * (CHUNK // 16)
            gather_ctr = 0
            prev_gather = None
            for _ in range(repeats):
                # one big contiguous load of every chunk's wrapped indices
                idx_all = idx_pool.tile([P, IDXW], mybir.dt.int16, tag="idx")
                nc.sync.dma_start(
                    out=idx_all[:],
                    in_=idx_t.ap()[:].rearrange("p nc two w -> p (nc two w)"),
                )
                # all scores accumulate on-chip; one big store at the end
                score_all = sc_pool.tile([P, TOTCAP // P], mybir.dt.float32,
                                         tag="score")
                for ci, (off, c, sb, db) in enumerate(CHUNKS):
                    cw = c // 16     # wrapped idx cols
                    cbase = ci * 2 * (CHUNK // 16)
                    # dma_gather is limited to 1024 indices per call (64
                    # descriptors per SDMA engine = one packet). Work in
                    # 1024-edge groups — two gathers -> mul -> reduce — so
                    # each DVE op depends on just two DMAs and overlap stays
                    # tight.
                    done = 0
                    while done < c:
                        g = min(GMAX, c - done)
                        ts = gat_pool.tile([P, GMAX], mybir.dt.float32,
                                           tag="ts")
                        td = gat_pool.tile([P, GMAX], mybir.dt.float32,
                                           tag="td")
                        for gt, bank, base in (
                            (ts, sb, cbase),
                            (td, db, cbase + CHUNK // 16),
                        ):
                            gi = nc.gpsimd.dma_gather(
                                out_ap=gt[:, :g].rearrange(
                                    "p (g d) -> p g d", d=D_FEAT
                                ),
                                in_ap=h_t.ap()[
                                    bank * BANK_SIZE : bank * BANK_SIZE
                                    + BANK_ROWS[bank]
                                ],
                                idxs_ap=idx_all[
                                    :, base + done // 16 : base + (done + g) // 16
                                ],
                                num_idxs=g,
                                num_idxs_reg=g,
                                elem_size=D_FEAT,
                                queue_num=gather_ctr % 4,
                            )
                            # Pin gather issue order = program order so the
                            # scheduler's DMASW lane rotation (8 lanes, by
                            # scheduled Pool-DMA order) stays aligned with the
                            # queue rotation (4 queues, program order) — a
                            # lane may only ever be updated from one queue.
                            if prev_gather is not None:
                                add_dep_helper(gi.ins, prev_gather.ins,
                                               sync=False)
                            prev_gather = gi
                            gather_ctr += 1
                        nc.vector.tensor_mul(
                            out=ts[:, :g], in0=ts[:, :g], in1=td[:, :g]
                        )
                        nc.vector.tensor_reduce(
                            out=score_all[
                                :, (off + done) // 128 : (off + done + g) // 128
                            ],
                            in_=ts[:, :g].rearrange("p (g d) -> p g d", d=D_FEAT),
                            axis=mybir.AxisListType.X,
                            op=mybir.AluOpType.add,
                        )
                        done += g
                nc.scalar.dma_start(out=out_t.ap()[:], in_=score_all[:])

    nc.compile()
    _build_cache[repeats] = nc
    return nc


def _wrap_idx(a):
    """[c] int16 -> [128, c/16]: idx i at [i%16, i//16], replicated x8."""
    w = a.reshape(-1, 16).T  # [16, c/16]
    return np.tile(w, (8, 1))


def _pack_core_inputs(h32, src, dst, core):
    """Bucket-sort this core's edges by (src_bank, dst_bank); build the
    device idx tensor and the inverse mapping for unpermuting scores.

    Returns (in_map, sorted_pos[E_CORE] int64 (-1 => overflow), overflow
    edge list (orig core-local positions)).
    """
    lo = core * E_CORE
    s = src[lo : lo + E_CORE]
    d = dst[lo : lo + E_CORE]
    sb = s >> BANK_SHIFT
    db = d >> BANK_SHIFT
    bucket = (sb * N_BANKS + db).astype(np.int64)
    order = np.argsort(bucket, kind="stable")
    sizes = np.bincount(bucket, minlength=N_BANKS * N_BANKS)

    sidx_sorted = np.zeros(TOTCAP, np.int16)
    didx_sorted = np.zeros(TOTCAP, np.int16)
    sorted_pos = np.full(E_CORE, -1, np.int64)
    overflow = []
    pos = 0
    for b in range(N_BANKS * N_BANKS):
        n = int(sizes[b])
        take = min(n, CAPS[b])
        sel = order[pos : pos + take]
        base = int(CAP_BASE[b])
        sidx_sorted[base : base + take] = (s[sel] & (BANK_SIZE - 1)).astype(
            np.int16
        )
        didx_sorted[base : base + take] = (d[sel] & (BANK_SIZE - 1)).astype(
            np.int16
        )
        sorted_pos[sel] = base + np.arange(take)
        if n > take:
            overflow.extend(order[pos + take : pos + n].tolist())
        pos += n

    idx_arr = np.zeros((N_CHUNKS, 2, P, CHUNK // 16), np.int16)
    for ci, (off, c, _sb, _db) in enumerate(CHUNKS):
        idx_arr[ci, 0, :, : c // 16] = _wrap_idx(sidx_sorted[off : off + c])
        idx_arr[ci, 1, :, : c // 16] = _wrap_idx(didx_sorted[off : off + c])
    # device wants partition-major: [P, N_CHUNKS, 2, CHUNK//16]
    idx_arr = np.ascontiguousarray(idx_arr.transpose(2, 0, 1, 3))

    return {"h": h32, "idx": idx_arr}, sorted_pos, overflow


def kernel(h, src, dst):
    from concourse.bass_utils import run_bass_kernel_spmd

    nc = _build()
    h32 = np.ascontiguousarray(np.asarray(h, dtype=np.float32))
    src64 = np.asarray(src).astype(np.int64)
    dst64 = np.asarray(dst).astype(np.int64)

    packed = [_pack_core_inputs(h32, src64, dst64, c) for c in range(N_CORES)]
    in_maps = [p[0] for p in packed]
    res = run_bass_kernel_spmd(nc, in_maps, core_ids=list(range(N_CORES)))

    out = np.empty(N_EDGES, np.float32)
    for c in range(N_CORES):
        _, sorted_pos, overflow = packed[c]
        # device layout [P, TOTCAP//P]: sorted index s = j*128 + p -> [p, j]
        scores_sorted = res.results[c]["score"].T.reshape(-1)
        oc = out[c * E_CORE : (c + 1) * E_CORE]
        valid = sorted_pos >= 0
        oc[valid] = scores_sorted[sorted_pos[valid]]
        if overflow:
            ov = np.asarray(overflow, np.int64)
            gs = src64[c * E_CORE + ov]
            gd = dst64[c * E_CORE + ov]
            oc[ov] = np.einsum("ed,ed->e", h32[gs], h32[gd])
    return out
